# revision 1
# baseline (speedup 1.0000x reference)
"""Causal attention (B=4, S=2048, D=1024, fp32 in/out) on 8 Trainium2 cores.

Sharding: core c = (batch b = c//2, variant h = c%2). Each core computes the
attention output for 1024 of the 2048 query rows of one batch element.

Load balancing ("sorted-slot" assignment): variant A owns global q-tiles
(0,1,2,3,12,13,14,15), variant B owns (4..11).  Slot i on every core
processes keys [0, CNT[i]*128) with CNT = (5,6,7,8,13,14,15,16), which
dominates both variants' causal needs, so a single NEFF (identical loop
structure) serves all 8 cores; per-core differences are carried entirely by
input data (pre-sliced/pre-transposed X, packed additive mask).

K/V are not recomputed per core: core (b, h) projects K^T/V only for its
own key half [h*1024, (h+1)*1024), and the pair exchanges halves with an
AllGather over replica groups [[0,1],[2,3],[4,5],[6,7]] through DRAM
bounce buffers (collectives can't touch I/O tensors).

Matmul operands are bf16 (fp32 matmul runs at half rate on the PE and
disables FWL); accumulation stays fp32 in PSUM and the softmax runs in
fp32, so the result error stays at the ~1e-3 level.

Per-core kernel:
  Phase A: Q^T = (Wq/32)^T Xq^T ; K^T_loc = Wk^T X_loc^T ; V_loc = X_loc Wv;
           AllGather K^T/V halves into SBUF-resident kT / v_sb (bf16).
  Phase B: per q-tile slot: S = Q^T.T K^T (PSUM fp32, accum over e), +mask
           on eviction (DVE, fp32), exp -> bf16 with fused fp32 row-sum
           (ScalarE accum_out), PE transpose of P tiles, O = P^T.T V
           accumulated in PSUM over key tiles, normalized by 1/rowsum on
           eviction (ScalarE Copy with scale=AP).

No max-subtraction in softmax: logits are (q.k)/32 with std ~0.33, bounded
by ~+-2, so exp() is perfectly conditioned; masked entries use -1e4.
"""

import numpy as np
from contextlib import ExitStack

import ml_dtypes

import concourse.bass as bass
import concourse.tile as tile
from concourse import bacc, mybir
from concourse.bass_utils import run_bass_kernel_spmd

P = 128
B, S, D = 4, 2048, 1024
NCORES = 8
DT = D // P      # 8 contraction tiles
ST = S // P      # 16 key tiles (global)
SLOC = S // 2    # 1024 local keys per core
SLT = SLOC // P  # 8 local key tiles
ET = D // P      # 8 output-feature tiles
QLOC = 1024      # query rows per core
QT = QLOC // P   # 8 local q tiles

G_A = (0, 1, 2, 3, 12, 13, 14, 15)   # variant A global q-tiles (slot order)
G_B = (4, 5, 6, 7, 8, 9, 10, 11)     # variant B
CNT = (5, 6, 7, 8, 13, 14, 15, 16)   # key tiles per slot (shared structure)
# Scores are computed transposed (S^T[k, q], keys on partitions).  Because
# CNT is ascending, the slots active for key-tile kt form a contiguous
# q-suffix starting at slot JKT[kt]; WKT[kt] is that suffix's width.
JKT = tuple(next(i for i in range(QT) if CNT[i] > kt) for kt in range(ST))
WKT = tuple((QT - j) * P for j in JKT)
OFFKT = tuple(int(x) for x in np.cumsum((0,) + WKT)[:-1])
MASK_COLS = sum(WKT)                 # 10752
NEG = -10000.0

F32 = mybir.dt.float32
BF16 = mybir.dt.bfloat16

REPLICA_GROUPS = [[0, 1], [2, 3], [4, 5], [6, 7]]


def _chunks(width, step=512):
    out = []
    c0 = 0
    while c0 < width:
        out.append((c0, min(step, width - c0)))
        c0 += out[-1][1]
    return out


def _build(reps=1):
    nc = bacc.Bacc("TRN2", target_bir_lowering=False, debug=False,
                   num_devices=NCORES)
    xt_in = nc.dram_tensor("xt", [D, SLOC], BF16, kind="ExternalInput").ap()
    xqt_in = nc.dram_tensor("xqt", [D, QLOC], BF16, kind="ExternalInput").ap()
    wq_in = nc.dram_tensor("wq", [D, D], BF16, kind="ExternalInput").ap()
    wk_in = nc.dram_tensor("wk", [D, D], BF16, kind="ExternalInput").ap()
    wv_in = nc.dram_tensor("wv", [D, D], BF16, kind="ExternalInput").ap()
    mask_in = nc.dram_tensor("mask", [P, MASK_COLS], BF16,
                             kind="ExternalInput").ap()
    out = nc.dram_tensor("out", [QLOC, D], F32, kind="ExternalOutput").ap()

    with tile.TileContext(nc) as tc, ExitStack() as ctx:
        persist = ctx.enter_context(tc.tile_pool(name="persist", bufs=1))
        kT = persist.tile([P, ET, S], BF16, tag="kT")      # K^T [e%128, et, key]
        qT = persist.tile([P, ET, QLOC], BF16, tag="qT")   # Q^T [e%128, et, q]
        v_sb = persist.tile([P, ST, D], BF16, tag="v")     # V   [k%128, kt, e]
        ones = persist.tile([P, 1], BF16, tag="ones")
        nc.gpsimd.memset(ones[:], 1.0)

        for _rep in range(reps):
            _emit_body(nc, tc, _rep, xt_in, xqt_in, wq_in, wk_in, wv_in,
                       mask_in, out, kT, qT, v_sb, ones)
    nc.compile()
    return nc


def _emit_body(nc, tc, rep, xt_in, xqt_in, wq_in, wk_in, wv_in, mask_in, out,
               kT, qT, v_sb, ones):
    body = ExitStack()
    # Masks are pure inputs; prefetch them from phase A so score evictions
    # (DVE add) never stall on mask arrival and back up PSUM.
    mpool = body.enter_context(tc.tile_pool(name="m", bufs=6))
    masks = []

    def _load_mask(kt):
        w = WKT[kt]
        m_t = mpool.tile([P, 8 * P], BF16, tag="m", name="m_t")[:, :w]
        nc.scalar.dma_start(m_t, mask_in[:, OFFKT[kt]:OFFKT[kt] + w])
        masks.append(m_t)

    def _prefetch_masks():
        # Only as many as the pool holds without waiting -- a waiting DMA
        # would head-of-line-block the ACT queue (kbounce/vbounce follow).
        for kt in range(6):
            _load_mask(kt)

    # ---------------- Phase A : projections + KV exchange ----------------
    with ExitStack() as pa:
        xp = pa.enter_context(tc.tile_pool(name="xp", bufs=1))
        dp = pa.enter_context(tc.tile_pool(name="dp", bufs=1, space="DRAM"))
        psA = pa.enter_context(tc.tile_pool(name="psA", bufs=8, space="PSUM"))

        # Critical-path inputs (K-proj needs wk+xt) on the SP queue, split
        # per d-tile so the first matmul starts as soon as slice 0 lands;
        # the rest stream on the ACT queue in parallel.
        xt = xp.tile([P, DT, SLOC], BF16, tag="xt")
        wq_t = xp.tile([P, DT, D], BF16, tag="wq")
        wk_t = xp.tile([P, DT, D], BF16, tag="wk")
        wv_t = xp.tile([P, DT, D], BF16, tag="wv")
        xqt = xp.tile([P, DT, QLOC], BF16, tag="xqt")
        for dt in range(DT):
            nc.sync.dma_start(wk_t[:, dt, :], wk_in[dt * P:(dt + 1) * P, :])
            nc.sync.dma_start(xt[:, dt, :], xt_in[dt * P:(dt + 1) * P, :])
        for dt in range(DT):
            nc.scalar.dma_start(wv_t[:, dt, :], wv_in[dt * P:(dt + 1) * P, :])
        for dt in range(DT):
            nc.scalar.dma_start(xqt[:, dt, :], xqt_in[dt * P:(dt + 1) * P, :])
        for dt in range(DT):
            nc.scalar.dma_start(wq_t[:, dt, :], wq_in[dt * P:(dt + 1) * P, :])
        _prefetch_masks()

        # Tiny warm-up collective: absorbs the cc firmware's first-use setup
        # latency while the input DMAs stream, so the real K gather is fast.
        warm_in = dp.tile([P, 8], BF16, tag="warm_in")
        warm_out = dp.tile([2 * P, 8], BF16, tag="warm_out")
        nc.gpsimd.collective_compute(
            "AllGather", mybir.AluOpType.bypass,
            replica_groups=REPLICA_GROUPS,
            ins=[warm_in.opt()], outs=[warm_out.opt()])

        klocal = xp.tile([P, ET, SLOC], BF16, tag="klocal")
        vlocal = xp.tile([P, SLT, D], BF16, tag="vlocal")
        kbounce = dp.tile([D, SLOC], BF16, tag="kbounce")
        kgather = dp.tile([2 * D, SLOC], BF16, tag="kgather")
        vbounce = dp.tile([SLOC, D], BF16, tag="vbounce")
        vgather = dp.tile([2 * SLOC, D], BF16, tag="vgather")

        # K^T_loc[et, k] = sum_d Wk[d, et].T X_loc^T[d, k]
        # dt is the OUTER loop (8 PSUM groups per half) so matmuls start as
        # soon as the first wk/xt slices land instead of waiting for all 8.
        for half in range(2):
            groups = [(et, kc) for et in range(half * 4, half * 4 + 4)
                      for kc in range(2)]
            pss = [psA.tile([P, 512], F32, tag="ps", name="ps")
                   for _ in groups]
            for dt in range(DT):
                for gi, (et, kc) in enumerate(groups):
                    nc.tensor.matmul(
                        pss[gi][:], lhsT=wk_t[:, dt, et * P:(et + 1) * P],
                        rhs=xt[:, dt, kc * 512:(kc + 1) * 512],
                        start=(dt == 0), stop=(dt == DT - 1))
            for gi, (et, kc) in enumerate(groups):
                nc.vector.tensor_copy(
                    klocal[:, et, kc * 512:(kc + 1) * 512], pss[gi][:])
        nc.scalar.dma_start(
            kbounce.rearrange("(et p) k -> p et k", p=P), klocal[:])
        nc.gpsimd.collective_compute(
            "AllGather", mybir.AluOpType.bypass,
            replica_groups=REPLICA_GROUPS,
            ins=[kbounce.opt()], outs=[kgather.opt()])
        # The SP queue is FIFO, so the bounce-out stores must not sit
        # behind these gather-dependent loads: stores go on the ACT queue.
        for et in range(ET):
            for r in range(2):
                nc.sync.dma_start(
                    kT[:, et, r * SLOC:(r + 1) * SLOC],
                    kgather[r * D + et * P: r * D + (et + 1) * P, :])

        # V_loc[kt, e] = sum_d X_loc^T[d, kt].T Wv[d, e]
        for half in range(2):
            groups = [(st, ec) for st in range(half * 4, half * 4 + 4)
                      for ec in range(2)]
            pss = [psA.tile([P, 512], F32, tag="ps", name="ps")
                   for _ in groups]
            for dt in range(DT):
                for gi, (st, ec) in enumerate(groups):
                    nc.tensor.matmul(
                        pss[gi][:], lhsT=xt[:, dt, st * P:(st + 1) * P],
                        rhs=wv_t[:, dt, ec * 512:(ec + 1) * 512],
                        start=(dt == 0), stop=(dt == DT - 1))
            for gi, (st, ec) in enumerate(groups):
                nc.vector.tensor_copy(
                    vlocal[:, st, ec * 512:(ec + 1) * 512], pss[gi][:])
        nc.scalar.dma_start(
            vbounce.rearrange("(st p) e -> p st e", p=P), vlocal[:])
        nc.gpsimd.collective_compute(
            "AllGather", mybir.AluOpType.bypass,
            replica_groups=REPLICA_GROUPS,
            ins=[vbounce.opt()], outs=[vgather.opt()])
        for kt in range(ST):
            nc.sync.dma_start(v_sb[:, kt, :],
                              vgather[kt * P:(kt + 1) * P, :])

        # Q^T[et, q] = sum_d Wq[d, et].T Xq^T[d, q]
        for half in range(2):
            groups = [(et, qc) for et in range(half * 4, half * 4 + 4)
                      for qc in range(2)]
            pss = [psA.tile([P, 512], F32, tag="ps", name="ps")
                   for _ in groups]
            for dt in range(DT):
                for gi, (et, qc) in enumerate(groups):
                    nc.tensor.matmul(
                        pss[gi][:], lhsT=wq_t[:, dt, et * P:(et + 1) * P],
                        rhs=xqt[:, dt, qc * 512:(qc + 1) * 512],
                        start=(dt == 0), stop=(dt == DT - 1))
            for gi, (et, qc) in enumerate(groups):
                nc.vector.tensor_copy(
                    qT[:, et, qc * 512:(qc + 1) * 512], pss[gi][:])

    # ---------------- Phase B : attention (transposed scores) ----------
    # S^T[k, q] with keys on partitions: slot layouts make the active slots
    # for key-tile kt a contiguous q-suffix, so one PSUM strip per kt.
    # exp(S^T) directly yields P^T -- the AV stationary operand -- with no
    # PE transposes; row-sums come from a ones-vector matmul fused into
    # the AV weight loads.
    with body, ExitStack() as pb:
        stile = pb.enter_context(tc.tile_pool(name="st", bufs=1))
        sT = stile.tile([P, ST, QLOC], F32, tag="sT")   # S^T [k%128, kt, q]
        # per-slot P^T tiles so an early slot's AV only waits its own exp
        ptpool = pb.enter_context(tc.tile_pool(name="pt", bufs=QT))
        opool = pb.enter_context(tc.tile_pool(name="o", bufs=2))
        stpool = pb.enter_context(tc.tile_pool(name="stat", bufs=QT))
        psS = pb.enter_context(tc.tile_pool(name="psS", bufs=2, space="PSUM"))
        psAV = pb.enter_context(tc.tile_pool(name="psAV", bufs=3, space="PSUM"))
        psRS = pb.enter_context(tc.tile_pool(name="psRS", bufs=1, space="PSUM"))

        pTs = {}
        for kt in range(ST):
            jq = JKT[kt] * P
            w = WKT[kt]
            if kt >= 6:
                _load_mask(kt)
            ps = psS.tile([P, 8 * P], F32, tag="psS", name="ps")[:, :w]
            for et in range(ET):
                for c0, cw in _chunks(w):
                    nc.tensor.matmul(
                        ps[:, c0:c0 + cw],
                        lhsT=kT[:, et, kt * P:(kt + 1) * P],
                        rhs=qT[:, et, jq + c0:jq + c0 + cw],
                        start=(et == 0), stop=(et == ET - 1))
            nc.vector.tensor_tensor(
                sT[:, kt, jq:QLOC], ps[:, :w], masks[kt][:, :w],
                op=mybir.AluOpType.add)
            # fire exp for every slot whose last key-tile this was
            for i in range(QT):
                if CNT[i] == kt + 1:
                    pT_i = ptpool.tile([P, ST, P], BF16, tag="pt", name="pT_i")
                    nc.scalar.activation(
                        pT_i[:, 0:CNT[i], :],
                        sT[:, 0:CNT[i], i * P:(i + 1) * P],
                        mybir.ActivationFunctionType.Exp)
                    pTs[i] = pT_i

        for i in range(QT):
            ck = CNT[i]
            pT_i = pTs[i]
            psavs = [psAV.tile([P, 512], F32, tag="psAV", name="psavs")
                     for _ in range(2)]
            psrs = psRS.tile([P, 1], F32, tag="psRS", name="psrs")
            for kt in range(ck):
                lhsT = pT_i[:, kt, :]
                nc.tensor.matmul(psrs[:], lhsT=lhsT, rhs=ones[:],
                                 start=(kt == 0), stop=(kt == ck - 1))
                for ec in range(2):
                    nc.tensor.matmul(
                        psavs[ec][:], lhsT=lhsT,
                        rhs=v_sb[:, kt, ec * 512:(ec + 1) * 512],
                        start=(kt == 0), stop=(kt == ck - 1))

            recip = stpool.tile([P, 1], F32, tag="rc", name="recip")
            nc.vector.reciprocal(recip[:], psrs[:])
            for ec in range(2):
                o_t = opool.tile([P, 512], F32, tag="o", name="o_t")
                nc.scalar.activation(o_t[:], psavs[ec][:],
                                     mybir.ActivationFunctionType.Copy,
                                     scale=recip[:])
                nc.sync.dma_start(
                    out[i * P:(i + 1) * P, ec * 512:(ec + 1) * 512], o_t[:])


_COMPILED = None


def _get_compiled():
    global _COMPILED
    if _COMPILED is None:
        _COMPILED = _build()
    return _COMPILED


def _qrows(G):
    return np.concatenate([np.arange(g * P, (g + 1) * P) for g in G])


def _host_mask(G):
    # Transposed additive mask: for key-tile kt the active slots are the
    # q-suffix JKT[kt]..7; column c = (slot_index - JKT[kt])*128 + q_in_tile,
    # row r = key_in_tile.  0 where key <= query position, NEG otherwise.
    m = np.full((P, MASK_COLS), NEG, np.float32)
    for kt in range(ST):
        key = kt * P + np.arange(P)[:, None]
        qpos = np.concatenate(
            [G[i] * P + np.arange(P) for i in range(JKT[kt], QT)])[None, :]
        m[:, OFFKT[kt]:OFFKT[kt] + WKT[kt]] = np.where(
            key <= qpos, np.float32(0.0), np.float32(NEG))
    return m.astype(ml_dtypes.bfloat16)


def _host_in_maps(X, Wq, Wk, Wv):
    bf = ml_dtypes.bfloat16
    X = np.asarray(X, np.float32)
    wq_s = (np.asarray(Wq, np.float32) / np.float32(np.sqrt(D))).astype(bf)
    wk = np.asarray(Wk, np.float32).astype(bf)
    wv = np.asarray(Wv, np.float32).astype(bf)
    masks = {0: _host_mask(G_A), 1: _host_mask(G_B)}
    qr = {0: _qrows(G_A), 1: _qrows(G_B)}
    in_maps = []
    for c in range(NCORES):
        b, h = divmod(c, 2)
        Xb = X[b]
        in_maps.append({
            "xt": np.ascontiguousarray(Xb[h * SLOC:(h + 1) * SLOC].T).astype(bf),
            "xqt": np.ascontiguousarray(Xb[qr[h]].T).astype(bf),
            "wq": wq_s, "wk": wk, "wv": wv,
            "mask": masks[h],
        })
    return in_maps, qr


def kernel(X, Wq, Wk, Wv, _trace=False):
    nc = _get_compiled()
    in_maps, qr = _host_in_maps(X, Wq, Wk, Wv)
    res = run_bass_kernel_spmd(nc, in_maps, core_ids=list(range(NCORES)),
                               trace=_trace)
    O = np.empty((B, S, D), np.float32)
    for c in range(NCORES):
        b, h = divmod(c, 2)
        O[b, qr[h]] = res.results[c]["out"]
    if _trace:
        kernel._last_exec_time_ns = res.exec_time_ns
        kernel._last_results = res
    return O



# revision 3
# speedup vs baseline: 1.1474x; 1.1474x over previous
"""Causal attention (B=4, S=2048, D=1024, fp32 in/out) on 8 Trainium2 cores.

Sharding: core c = (batch b = c//2, variant h = c%2). Each core computes the
attention output for 1024 of the 2048 query rows of one batch element.

Load balancing ("parity-slot" assignment): variant A owns even global
q-tiles (0,2,...,14), variant B owns odd (1,3,...,15). Slot i on every core
processes keys [0, CNT[i]*128) with CNT = (2,4,6,...,16), which dominates
both variants' causal needs (72 key-tiles vs the 68 minimum), so a single
NEFF serves all 8 cores; per-core differences are carried entirely by input
data (pre-sliced/pre-transposed X, packed additive mask).

K/V are not recomputed per core: core (b, h) projects K^T/V only for its
own key half [h*1024, (h+1)*1024), and the pair exchanges halves with
chunked AllGathers over replica groups [[0,1],[2,3],[4,5],[6,7]] through
DRAM bounce buffers, pipelined so early key tiles land in SBUF while later
projection halves still compute.

Numerics: projections and AV run in bf16 (fp32 PSUM accum). Scores run in
fp8e4 (e4m3) with DoubleRow perf mode - each matmul contracts TWO 128-e
tiles into 64 psum partitions at 0.5 cycles/column, 2x bf16 throughput.
Q^T/K^T are cast fp32->fp8 at projection eviction; the 1/sqrt(1024) logit
scale is applied inside the exp activation (scale=1/32), so q/k stay at
full range (|q| ~ 0.6 std) where e4m3 quantization is benign. Measured
end-to-end rel err ~1.3e-2 (vs 2e-2 budget).

DoubleRow cannot target PSUM partition offset 64 (invalid ISA), so the two
64-key groups of a strip go to separate [64, .] psum tiles; the DVE
mask-add eviction writes group 1 to sT partitions 64:128 directly (engines
honor per-operand partition bases).

Per-core kernel:
  Phase A: K^T_loc = Wk^T X_loc^T per 512-key half (chunked store+gather);
           V_loc = X_loc Wv per 256-row quarter (chunked store+gather);
           Q^T = Wq^T Xq^T; evictions cast K/Q to fp8, V to bf16.
  Phase B: per key-tile kt: S^T strip via fp8 DoubleRow (4 et-pair matmuls
           x 2 key-groups per 512-col chunk), +mask on eviction (DVE,
           fp32), exp (scale=1/32) -> bf16 pT per finished slot; AV
           accumulates O = P^T.T V in PSUM over key tiles with a fused
           ones-matmul rowsum column, normalized on eviction (ScalarE Copy
           with scale=1/rowsum).
"""

import numpy as np
from contextlib import ExitStack

import ml_dtypes

import concourse.bass as bass
import concourse.tile as tile
from concourse import bacc, mybir
from concourse.bass_utils import run_bass_kernel_spmd

P = 128
B, S, D = 4, 2048, 1024
NCORES = 8
DT = D // P      # 8 contraction tiles
ST = S // P      # 16 key tiles (global)
SLOC = S // 2    # 1024 local keys per core
SLT = SLOC // P  # 8 local key tiles
ET = D // P      # 8 output-feature tiles
QLOC = 1024      # query rows per core
QT = QLOC // P   # 8 local q tiles

G_A = tuple(range(0, ST, 2))         # variant A global q-tiles (slot order)
G_B = tuple(range(1, ST, 2))         # variant B
CNT = tuple(2 * i + 2 for i in range(QT))  # key tiles per slot (shared)
# Scores are computed transposed (S^T[k, q], keys on partitions).  Because
# CNT is ascending, the slots active for key-tile kt form a contiguous
# q-suffix starting at slot JKT[kt]; WKT[kt] is that suffix's width.
JKT = tuple(next(i for i in range(QT) if CNT[i] > kt) for kt in range(ST))
WKT = tuple((QT - j) * P for j in JKT)
OFFKT = tuple(int(x) for x in np.cumsum((0,) + WKT)[:-1])
MASK_COLS = sum(WKT)                 # 9216
NEG = -10000.0
INV_SQRT_D = 1.0 / 32.0

F32 = mybir.dt.float32
BF16 = mybir.dt.bfloat16
F8 = mybir.dt.float8e4
DR = mybir.MatmulPerfMode.DoubleRow

REPLICA_GROUPS = [[0, 1], [2, 3], [4, 5], [6, 7]]


def _chunks(width, step=512):
    out = []
    c0 = 0
    while c0 < width:
        out.append((c0, min(step, width - c0)))
        c0 += out[-1][1]
    return out


def _build(reps=1):
    nc = bacc.Bacc("TRN2", target_bir_lowering=False, debug=False,
                   num_devices=NCORES)
    xt_in = nc.dram_tensor("xt", [D, SLOC], BF16, kind="ExternalInput").ap()
    xqt_in = nc.dram_tensor("xqt", [D, QLOC], BF16, kind="ExternalInput").ap()
    wq_in = nc.dram_tensor("wq", [D, D], BF16, kind="ExternalInput").ap()
    wk_in = nc.dram_tensor("wk", [D, D], BF16, kind="ExternalInput").ap()
    wv_in = nc.dram_tensor("wv", [D, D], BF16, kind="ExternalInput").ap()
    mask_in = nc.dram_tensor("mask", [P, MASK_COLS], BF16,
                             kind="ExternalInput").ap()
    out = nc.dram_tensor("out", [QLOC, D], F32, kind="ExternalOutput").ap()

    with tile.TileContext(nc) as tc, ExitStack() as ctx:
        persist = ctx.enter_context(tc.tile_pool(name="persist", bufs=1))
        kT = persist.tile([P, ET, S], F8, tag="kT")         # K^T [e%128, et, key]
        qT = persist.tile([P, ET, QLOC], F8, tag="qT")      # Q^T [e%128, et, q]
        v_sb = persist.tile([P, ST, D], BF16, tag="v")      # V   [k%128, kt, e]
        ones = persist.tile([P, 1], BF16, tag="ones")

        for _rep in range(reps):
            _emit_body(nc, tc, _rep, xt_in, xqt_in, wq_in, wk_in, wv_in,
                       mask_in, out, kT, qT, v_sb, ones)
    nc.compile()
    return nc


def _emit_body(nc, tc, rep, xt_in, xqt_in, wq_in, wk_in, wv_in, mask_in, out,
               kT, qT, v_sb, ones):
    body = ExitStack()
    # Masks are pure inputs; prefetch them from phase A so score evictions
    # (DVE add) never stall on mask arrival and back up PSUM.
    mpool = body.enter_context(tc.tile_pool(name="m", bufs=6))
    masks = []

    def _load_mask(kt):
        w = WKT[kt]
        m_t = mpool.tile([P, 8 * P], BF16, tag="m", name="m_t")[:, :w]
        nc.sync.dma_start(m_t, mask_in[:, OFFKT[kt]:OFFKT[kt] + w])
        masks.append(m_t)

    # ---------------- Phase A : projections + KV exchange ----------------
    with ExitStack() as pa:
        xp = pa.enter_context(tc.tile_pool(name="xp", bufs=1))
        dp = pa.enter_context(tc.tile_pool(name="dp", bufs=1, space="DRAM"))
        psA = pa.enter_context(tc.tile_pool(name="psA", bufs=8, space="PSUM"))

        # Tiny warm-up collective first: absorbs the cc firmware's setup
        # latency (~40us) while the input DMAs stream.
        warm_in = dp.tile([P, 8], BF16, tag="warm_in")
        warm_out = dp.tile([2 * P, 8], BF16, tag="warm_out")
        nc.gpsimd.collective_compute(
            "AllGather", mybir.AluOpType.bypass,
            replica_groups=REPLICA_GROUPS,
            ins=[warm_in.opt()], outs=[warm_out.opt()])
        nc.gpsimd.memset(ones[:], 1.0)

        # K-proj inputs (wk+xt) split across BOTH DMA queues so the first
        # matmul starts after ~0.5MB and per-dt delivery outpaces the PE.
        xt = xp.tile([P, DT, SLOC], BF16, tag="xt")
        wq_t = xp.tile([P, DT, D], BF16, tag="wq")
        wk_t = xp.tile([P, DT, D], BF16, tag="wk")
        wv_t = xp.tile([P, DT, D], BF16, tag="wv")
        xqt = xp.tile([P, DT, QLOC], BF16, tag="xqt")
        for dt in range(DT):
            nc.sync.dma_start(wk_t[:, dt, :], wk_in[dt * P:(dt + 1) * P, :])
            nc.scalar.dma_start(xt[:, dt, :], xt_in[dt * P:(dt + 1) * P, :])
        for dt in range(DT):
            nc.sync.dma_start(wv_t[:, dt, :], wv_in[dt * P:(dt + 1) * P, :])
        for dt in range(DT):
            nc.scalar.dma_start(xqt[:, dt, :], xqt_in[dt * P:(dt + 1) * P, :])
        for dt in range(DT):
            nc.scalar.dma_start(wq_t[:, dt, :], wq_in[dt * P:(dt + 1) * P, :])
        for kt in range(6):
            _load_mask(kt)

        klocal = xp.tile([P, ET, SLOC], F8, tag="klocal")
        vlocal = xp.tile([P, SLT, D], BF16, tag="vlocal")
        kbounce = [dp.tile([D, 512], F8, tag="kb", name=f"kb{c}")
                   for c in range(2)]
        kgather = [dp.tile([2 * D, 512], F8, tag="kg", name=f"kg{c}")
                   for c in range(2)]
        vbounce = [dp.tile([2 * P, D], BF16, tag="vb", name=f"vb{c}")
                   for c in range(4)]
        vgather = [dp.tile([4 * P, D], BF16, tag="vg", name=f"vg{c}")
                   for c in range(4)]

        # K^T_loc[et, k] = sum_d Wk[d, et].T X_loc^T[d, k].  Halves are key
        # chunks (kc), so chunk kc's store+gather overlaps the other half's
        # matmuls; dt is the outer loop so matmuls start as slices land.
        for kc in range(2):
            pss = [psA.tile([P, 512], F32, tag="ps", name="ps")
                   for _ in range(ET)]
            for dt in range(DT):
                for et in range(ET):
                    nc.tensor.matmul(
                        pss[et][:], lhsT=wk_t[:, dt, et * P:(et + 1) * P],
                        rhs=xt[:, dt, kc * 512:(kc + 1) * 512],
                        start=(dt == 0), stop=(dt == DT - 1))
            for et in range(ET):
                nc.vector.tensor_copy(
                    klocal[:, et, kc * 512:(kc + 1) * 512], pss[et][:])
            nc.gpsimd.dma_start(
                kbounce[kc].rearrange("(et p) k -> p et k", p=P),
                klocal[:, :, kc * 512:(kc + 1) * 512])
            nc.gpsimd.collective_compute(
                "AllGather", mybir.AluOpType.bypass,
                replica_groups=REPLICA_GROUPS,
                ins=[kbounce[kc].opt()], outs=[kgather[kc].opt()])
            # Gather-dependent loads go on the scalar queue (idle once the
            # inputs are streamed); nothing later must pass them.
            for r in range(2):
                for et in range(ET):
                    nc.scalar.dma_start(
                        kT[:, et, r * SLOC + kc * 512:r * SLOC + (kc + 1) * 512],
                        kgather[kc][r * D + et * P:r * D + (et + 1) * P, :])

        # V_loc[kt, e] = sum_d X_loc^T[d, kt].T Wv[d, e]; store+gather per
        # 2-key-tile chunk so early v_sb tiles land well before AV needs them.
        for half in range(2):
            groups = [(st, ec) for st in range(half * 4, half * 4 + 4)
                      for ec in range(2)]
            pss = [psA.tile([P, 512], F32, tag="ps", name="ps")
                   for _ in groups]
            for dt in range(DT):
                for gi, (st, ec) in enumerate(groups):
                    nc.tensor.matmul(
                        pss[gi][:], lhsT=xt[:, dt, st * P:(st + 1) * P],
                        rhs=wv_t[:, dt, ec * 512:(ec + 1) * 512],
                        start=(dt == 0), stop=(dt == DT - 1))
            for gi, (st, ec) in enumerate(groups):
                nc.vector.tensor_copy(
                    vlocal[:, st, ec * 512:(ec + 1) * 512], pss[gi][:])
            for c in (2 * half, 2 * half + 1):
                nc.gpsimd.dma_start(
                    vbounce[c].rearrange("(st p) e -> p st e", p=P),
                    vlocal[:, 2 * c:2 * c + 2, :])
                nc.gpsimd.collective_compute(
                    "AllGather", mybir.AluOpType.bypass,
                    replica_groups=REPLICA_GROUPS,
                    ins=[vbounce[c].opt()], outs=[vgather[c].opt()])
                for r in range(2):
                    for j in range(2):
                        nc.scalar.dma_start(
                            v_sb[:, r * SLT + 2 * c + j, :],
                            vgather[c][(2 * r + j) * P:(2 * r + j + 1) * P, :])

        # Q^T[et, q] = sum_d Wq[d, et].T Xq^T[d, q]
        for half in range(2):
            groups = [(et, qc) for et in range(half * 4, half * 4 + 4)
                      for qc in range(2)]
            pss = [psA.tile([P, 512], F32, tag="ps", name="ps")
                   for _ in groups]
            for dt in range(DT):
                for gi, (et, qc) in enumerate(groups):
                    nc.tensor.matmul(
                        pss[gi][:], lhsT=wq_t[:, dt, et * P:(et + 1) * P],
                        rhs=xqt[:, dt, qc * 512:(qc + 1) * 512],
                        start=(dt == 0), stop=(dt == DT - 1))
            for gi, (et, qc) in enumerate(groups):
                nc.vector.tensor_copy(
                    qT[:, et, qc * 512:(qc + 1) * 512], pss[gi][:])

    # ---------------- Phase B : attention (transposed scores) ----------
    # S^T[k, q] with keys on partitions, fp8 DoubleRow: each matmul
    # contracts an et PAIR into 64 psum partitions (one 64-key group).
    # exp(S^T) directly yields P^T -- the AV stationary operand.
    with body, ExitStack() as pb:
        stile = pb.enter_context(tc.tile_pool(name="st", bufs=1))
        sT = stile.tile([P, ST, QLOC], F32, tag="sT")   # S^T [k%128, kt, q]
        # per-slot P^T tiles so an early slot's AV only waits its own exp
        ptpool = pb.enter_context(tc.tile_pool(name="pt", bufs=QT))
        opool = pb.enter_context(tc.tile_pool(name="o", bufs=2))
        stpool = pb.enter_context(tc.tile_pool(name="stat", bufs=QT))
        psS = pb.enter_context(tc.tile_pool(name="psS", bufs=2, space="PSUM"))
        psAV = pb.enter_context(tc.tile_pool(name="psAV", bufs=3, space="PSUM"))
        psRS = pb.enter_context(tc.tile_pool(name="psRS", bufs=1, space="PSUM"))
        rs = psRS.tile([P, QT], F32, tag="rs")          # rowsum, col per slot

        pTs = {}
        for kt in range(ST):
            jq = JKT[kt] * P
            w = WKT[kt]
            if kt >= 6:
                _load_mask(kt)
            for c0, cw in _chunks(w):
                ps = psS.tile([64, 2, 512], F32, tag="psS", name="ps")
                for kg in range(2):
                    for ep in range(ET // 2):
                        nc.tensor.matmul(
                            ps[:, kg, :cw],
                            lhsT=kT[:, 2 * ep:2 * ep + 2,
                                    kt * P + kg * 64:kt * P + kg * 64 + 64],
                            rhs=qT[:, 2 * ep:2 * ep + 2, jq + c0:jq + c0 + cw],
                            start=(ep == 0), stop=(ep == ET // 2 - 1),
                            perf_mode=DR)
                # mask-add eviction; kg1 lands on sT partitions 64:128
                for kg in range(2):
                    nc.vector.tensor_tensor(
                        sT[kg * 64:(kg + 1) * 64, kt, jq + c0:jq + c0 + cw],
                        ps[:, kg, :cw],
                        masks[kt][kg * 64:(kg + 1) * 64, c0:c0 + cw],
                        op=mybir.AluOpType.add)
            # fire exp for every slot whose last key-tile this was
            for i in range(QT):
                if CNT[i] == kt + 1:
                    pT_i = ptpool.tile([P, ST, P], BF16, tag="pt", name="pT_i")
                    nc.scalar.activation(
                        pT_i[:, 0:CNT[i], :],
                        sT[:, 0:CNT[i], i * P:(i + 1) * P],
                        mybir.ActivationFunctionType.Exp,
                        scale=INV_SQRT_D)
                    pTs[i] = pT_i

        for i in range(QT):
            ck = CNT[i]
            pT_i = pTs[i]
            psavs = [psAV.tile([P, 512], F32, tag="psAV", name="psavs")
                     for _ in range(2)]
            for kt in range(ck):
                lhsT = pT_i[:, kt, :]
                nc.tensor.matmul(rs[:, i:i + 1], lhsT=lhsT, rhs=ones[:],
                                 start=(kt == 0), stop=(kt == ck - 1))
                for ec in range(2):
                    nc.tensor.matmul(
                        psavs[ec][:], lhsT=lhsT,
                        rhs=v_sb[:, kt, ec * 512:(ec + 1) * 512],
                        start=(kt == 0), stop=(kt == ck - 1))

            recip = stpool.tile([P, 1], F32, tag="rc", name="recip")
            nc.vector.reciprocal(recip[:], rs[:, i:i + 1])
            for ec in range(2):
                o_t = opool.tile([P, 512], F32, tag="o", name="o_t")
                nc.scalar.activation(o_t[:], psavs[ec][:],
                                     mybir.ActivationFunctionType.Copy,
                                     scale=recip[:])
                nc.sync.dma_start(
                    out[i * P:(i + 1) * P, ec * 512:(ec + 1) * 512], o_t[:])


_COMPILED = None


def _get_compiled():
    global _COMPILED
    if _COMPILED is None:
        _COMPILED = _build()
    return _COMPILED


def _qrows(G):
    return np.concatenate([np.arange(g * P, (g + 1) * P) for g in G])


def _host_mask(G):
    # Transposed additive mask: for key-tile kt the active slots are the
    # q-suffix JKT[kt]..7; column c = (slot_index - JKT[kt])*128 + q_in_tile,
    # row r = key_in_tile.  0 where key <= query position, NEG otherwise.
    m = np.full((P, MASK_COLS), NEG, np.float32)
    for kt in range(ST):
        key = kt * P + np.arange(P)[:, None]
        qpos = np.concatenate(
            [G[i] * P + np.arange(P) for i in range(JKT[kt], QT)])[None, :]
        m[:, OFFKT[kt]:OFFKT[kt] + WKT[kt]] = np.where(
            key <= qpos, np.float32(0.0), np.float32(NEG))
    return m.astype(ml_dtypes.bfloat16)


def _host_in_maps(X, Wq, Wk, Wv):
    bf = ml_dtypes.bfloat16
    X = np.asarray(X, np.float32)
    wq = np.asarray(Wq, np.float32).astype(bf)
    wk = np.asarray(Wk, np.float32).astype(bf)
    wv = np.asarray(Wv, np.float32).astype(bf)
    masks = {0: _host_mask(G_A), 1: _host_mask(G_B)}
    qr = {0: _qrows(G_A), 1: _qrows(G_B)}
    in_maps = []
    for c in range(NCORES):
        b, h = divmod(c, 2)
        Xb = X[b]
        in_maps.append({
            "xt": np.ascontiguousarray(Xb[h * SLOC:(h + 1) * SLOC].T).astype(bf),
            "xqt": np.ascontiguousarray(Xb[qr[h]].T).astype(bf),
            "wq": wq, "wk": wk, "wv": wv,
            "mask": masks[h],
        })
    return in_maps, qr


def kernel(X, Wq, Wk, Wv, _trace=False):
    nc = _get_compiled()
    in_maps, qr = _host_in_maps(X, Wq, Wk, Wv)
    res = run_bass_kernel_spmd(nc, in_maps, core_ids=list(range(NCORES)),
                               trace=_trace)
    O = np.empty((B, S, D), np.float32)
    for c in range(NCORES):
        b, h = divmod(c, 2)
        O[b, qr[h]] = res.results[c]["out"]
    if _trace:
        kernel._last_exec_time_ns = res.exec_time_ns
        kernel._last_results = res
    return O


# revision 13
# speedup vs baseline: 1.1550x; 1.0067x over previous
"""Causal attention (B=4, S=2048, D=1024, fp32 in/out) on 8 Trainium2 cores.

Sharding: core c = (batch b = c//2, variant h = c%2). Each core computes the
attention output for 1024 of the 2048 query rows of one batch element.

Load balancing ("parity-slot" assignment): variant A owns even global
q-tiles (0,2,...,14), variant B owns odd (1,3,...,15). Slot i on every core
processes keys [0, CNT[i]*128) with CNT = (2,4,6,...,16), which dominates
both variants' causal needs (72 key-tiles vs the 68 minimum), so a single
NEFF serves all 8 cores; per-core differences are carried entirely by input
data (pre-sliced/pre-transposed X, packed additive mask).

K/V are not recomputed per core: core (b, h) projects K^T/V only for its
own key half [h*1024, (h+1)*1024), and the pair exchanges halves with
chunked AllGathers over replica groups [[0,1],[2,3],[4,5],[6,7]] through
DRAM bounce buffers, pipelined so early key tiles land in SBUF while later
projection halves still compute.

Numerics: projections and AV run in bf16 (fp32 PSUM accum). Scores run in
fp8e4 (e4m3) with DoubleRow perf mode - each matmul contracts TWO 128-e
tiles into 64 psum partitions at 0.5 cycles/column, 2x bf16 throughput.
Q^T/K^T are cast fp32->fp8 at projection eviction; the 1/sqrt(1024) logit
scale is applied inside the exp activation (scale=1/32), so q/k stay at
full range (|q| ~ 0.6 std) where e4m3 quantization is benign. Measured
end-to-end rel err ~1.3e-2 (vs 2e-2 budget).

DoubleRow cannot target PSUM partition offset 64 (invalid ISA), so the two
64-key groups of a strip go to separate [64, .] psum tiles; the DVE
mask-add eviction writes group 1 to sT partitions 64:128 directly (engines
honor per-operand partition bases).

Per-core kernel:
  Phase A: K^T_loc = Wk^T X_loc^T per 512-key half (chunked store+gather);
           V_loc = X_loc Wv per 256-row quarter (chunked store+gather);
           Q^T = Wq^T Xq^T; evictions cast K/Q to fp8, V to bf16.
  Phase B: per key-tile kt: S^T strip via fp8 DoubleRow (4 et-pair matmuls
           x 2 key-groups per 512-col chunk), +mask on eviction (DVE,
           fp32), exp (scale=1/32) -> bf16 pT per finished slot; AV
           accumulates O = P^T.T V in PSUM over key tiles with a fused
           ones-matmul rowsum column, normalized on eviction (ScalarE Copy
           with scale=1/rowsum).
"""

import numpy as np
from contextlib import ExitStack

import ml_dtypes

import concourse.bass as bass
import concourse.tile as tile
from concourse import bacc, mybir
from concourse.bass_utils import run_bass_kernel_spmd

P = 128
B, S, D = 4, 2048, 1024
NCORES = 8
DT = D // P      # 8 contraction tiles
ST = S // P      # 16 key tiles (global)
SLOC = S // 2    # 1024 local keys per core
SLT = SLOC // P  # 8 local key tiles
ET = D // P      # 8 output-feature tiles
QLOC = 1024      # query rows per core
QT = QLOC // P   # 8 local q tiles

G_A = tuple(range(0, ST, 2))         # variant A global q-tiles (slot order)
G_B = tuple(range(1, ST, 2))         # variant B
CNT = tuple(2 * i + 2 for i in range(QT))  # key tiles per slot (shared)
# Scores are computed transposed (S^T[k, q], keys on partitions).  Because
# CNT is ascending, the slots active for key-tile kt form a contiguous
# q-suffix starting at slot JKT[kt]; WKT[kt] is that suffix's width.
JKT = tuple(next(i for i in range(QT) if CNT[i] > kt) for kt in range(ST))
WKT = tuple((QT - j) * P for j in JKT)
OFFKT = tuple(int(x) for x in np.cumsum((0,) + WKT)[:-1])
MASK_COLS = sum(WKT)                 # 9216
NEG = -10000.0
INV_SQRT_D = 1.0 / 32.0
# Score strips 8..15 first (they only need the qc=1 half of Q^T), then
# 0..7.  Slots 0..3 finish at strip CNT[i]-1 in the second part; slots
# 4..7 need strips from both parts and all finish after strip 7.
STRIP_ORDER = tuple(range(8, ST)) + tuple(range(8))

F32 = mybir.dt.float32
BF16 = mybir.dt.bfloat16
F8 = mybir.dt.float8e4
DR = mybir.MatmulPerfMode.DoubleRow

REPLICA_GROUPS = [[0, 1], [2, 3], [4, 5], [6, 7]]


def _chunks(width, step=512):
    out = []
    c0 = 0
    while c0 < width:
        out.append((c0, min(step, width - c0)))
        c0 += out[-1][1]
    return out


def _build(reps=1):
    nc = bacc.Bacc("TRN2", target_bir_lowering=False, debug=False,
                   num_devices=NCORES)
    xt_in = nc.dram_tensor("xt", [D, SLOC], BF16, kind="ExternalInput").ap()
    xqt_in = nc.dram_tensor("xqt", [D, QLOC], BF16, kind="ExternalInput").ap()
    wq_in = nc.dram_tensor("wq", [D, D], BF16, kind="ExternalInput").ap()
    wk_in = nc.dram_tensor("wk", [D, D], BF16, kind="ExternalInput").ap()
    wv_in = nc.dram_tensor("wv", [D, D], BF16, kind="ExternalInput").ap()
    mask_in = nc.dram_tensor("mask", [P, MASK_COLS], BF16,
                             kind="ExternalInput").ap()
    out = nc.dram_tensor("out", [QLOC, D], F32, kind="ExternalOutput").ap()

    with tile.TileContext(nc) as tc, ExitStack() as ctx:
        persist = ctx.enter_context(tc.tile_pool(name="persist", bufs=1))
        kT = persist.tile([P, ET, S], F8, tag="kT")         # K^T [e%128, et, key]
        qT = persist.tile([P, ET, QLOC], F8, tag="qT")      # Q^T [e%128, et, q]
        v_sb = persist.tile([P, ST, D], BF16, tag="v")      # V   [k%128, kt, e]
        ones = persist.tile([P, 1], BF16, tag="ones")

        for _rep in range(reps):
            _emit_body(nc, tc, _rep, xt_in, xqt_in, wq_in, wk_in, wv_in,
                       mask_in, out, kT, qT, v_sb, ones)
    nc.compile()
    return nc


def _emit_body(nc, tc, rep, xt_in, xqt_in, wq_in, wk_in, wv_in, mask_in, out,
               kT, qT, v_sb, ones):
    body = ExitStack()
    # Masks are pure inputs; prefetch them from phase A so score evictions
    # (DVE add) never stall on mask arrival and back up PSUM.
    mpool = body.enter_context(tc.tile_pool(name="m", bufs=6))
    masks = {}

    def _load_mask(kt):
        w = WKT[kt]
        m_t = mpool.tile([P, 8 * P], BF16, tag="m", name="m_t")[:, :w]
        nc.sync.dma_start(m_t, mask_in[:, OFFKT[kt]:OFFKT[kt] + w])
        masks[kt] = m_t

    # ---------------- Phase A : projections + KV exchange ----------------
    with ExitStack() as pa:
        xp = pa.enter_context(tc.tile_pool(name="xp", bufs=1))
        dp = pa.enter_context(tc.tile_pool(name="dp", bufs=1, space="DRAM"))
        psA = pa.enter_context(tc.tile_pool(name="psA", bufs=8, space="PSUM"))

        # No warm-up collective: the runtime's collectives-init barrier
        # occupies the CC stream until ~40us regardless, so the first real
        # gather absorbs the firmware setup itself; a warm-up op would only
        # delay it by its own ~7us of CC-stream occupancy.
        nc.gpsimd.memset(ones[:], 1.0)

        # K-proj inputs (wk+xt) split across BOTH DMA queues so the first
        # matmul starts after ~0.5MB and per-dt delivery outpaces the PE.
        xt = xp.tile([P, DT, SLOC], BF16, tag="xt")
        wq_t = xp.tile([P, DT, D], BF16, tag="wq")
        wk_t = xp.tile([P, DT, D], BF16, tag="wk")
        wv_t = xp.tile([P, DT, D], BF16, tag="wv")
        xqt = xp.tile([P, DT, QLOC], BF16, tag="xqt")
        for dt in range(DT):
            nc.sync.dma_start(wk_t[:, dt, :], wk_in[dt * P:(dt + 1) * P, :])
            nc.scalar.dma_start(xt[:, dt, :], xt_in[dt * P:(dt + 1) * P, :])
        for dt in range(DT):
            nc.sync.dma_start(wv_t[:, dt, :], wv_in[dt * P:(dt + 1) * P, :])
        for dt in range(DT):
            nc.scalar.dma_start(xqt[:, dt, :], xqt_in[dt * P:(dt + 1) * P, :])
        for dt in range(DT):
            nc.scalar.dma_start(wq_t[:, dt, :], wq_in[dt * P:(dt + 1) * P, :])
        for kt in STRIP_ORDER[:6]:
            _load_mask(kt)

        klocal = xp.tile([P, ET, SLOC], F8, tag="klocal")
        vlocal = xp.tile([P, SLT, D], BF16, tag="vlocal")
        kbounce = [dp.tile([D, 512], F8, tag="kb", name=f"kb{c}")
                   for c in range(2)]
        kgather = [dp.tile([2 * D, 512], F8, tag="kg", name=f"kg{c}")
                   for c in range(2)]
        vbounce = [dp.tile([4 * P, D], BF16, tag="vb", name=f"vb{c}")
                   for c in range(2)]
        vgather = [dp.tile([8 * P, D], BF16, tag="vg", name=f"vg{c}")
                   for c in range(2)]

        # K^T_loc[et, k] = sum_d Wk[d, et].T X_loc^T[d, k].  Halves are key
        # chunks (kc), so chunk kc's store+gather overlaps the other half's
        # matmuls; dt is the outer loop so matmuls start as slices land.
        for kc in range(2):
            pss = [psA.tile([P, 512], F32, tag="ps", name="ps")
                   for _ in range(ET)]
            for dt in range(DT):
                for et in range(ET):
                    nc.tensor.matmul(
                        pss[et][:], lhsT=wk_t[:, dt, et * P:(et + 1) * P],
                        rhs=xt[:, dt, kc * 512:(kc + 1) * 512],
                        start=(dt == 0), stop=(dt == DT - 1))
            for et in range(ET):
                nc.vector.tensor_copy(
                    klocal[:, et, kc * 512:(kc + 1) * 512], pss[et][:])
            nc.gpsimd.dma_start(
                kbounce[kc].rearrange("(et p) k -> p et k", p=P),
                klocal[:, :, kc * 512:(kc + 1) * 512])
            nc.gpsimd.collective_compute(
                "AllGather", mybir.AluOpType.bypass,
                replica_groups=REPLICA_GROUPS,
                ins=[kbounce[kc].opt()], outs=[kgather[kc].opt()])
            # Gather-dependent loads go on the scalar queue (idle once the
            # inputs are streamed); nothing later must pass them.
            for r in range(2):
                for et in range(ET):
                    nc.scalar.dma_start(
                        kT[:, et, r * SLOC + kc * 512:r * SLOC + (kc + 1) * 512],
                        kgather[kc][r * D + et * P:r * D + (et + 1) * P, :])

        # V_loc[kt, e] = sum_d X_loc^T[d, kt].T Wv[d, e]; each half (4 local
        # key tiles) is one store+gather chunk, so chunk 0's exchange
        # overlaps half 1's matmuls and the Q projection.
        for half in range(2):
            groups = [(st, ec) for st in range(half * 4, half * 4 + 4)
                      for ec in range(2)]
            pss = [psA.tile([P, 512], F32, tag="ps", name="ps")
                   for _ in groups]
            for dt in range(DT):
                for gi, (st, ec) in enumerate(groups):
                    nc.tensor.matmul(
                        pss[gi][:], lhsT=xt[:, dt, st * P:(st + 1) * P],
                        rhs=wv_t[:, dt, ec * 512:(ec + 1) * 512],
                        start=(dt == 0), stop=(dt == DT - 1))
            for gi, (st, ec) in enumerate(groups):
                nc.vector.tensor_copy(
                    vlocal[:, st, ec * 512:(ec + 1) * 512], pss[gi][:])
            c = half
            nc.gpsimd.dma_start(
                vbounce[c].rearrange("(st p) e -> p st e", p=P),
                vlocal[:, 4 * c:4 * c + 4, :])
            nc.gpsimd.collective_compute(
                "AllGather", mybir.AluOpType.bypass,
                replica_groups=REPLICA_GROUPS,
                ins=[vbounce[c].opt()], outs=[vgather[c].opt()])
            for r in range(2):
                for j in range(4):
                    nc.scalar.dma_start(
                        v_sb[:, r * SLT + 4 * c + j, :],
                        vgather[c][(4 * r + j) * P:(4 * r + j + 1) * P, :])

        # Q^T[et, q] = sum_d Wq[d, et].T Xq^T[d, q].  Halves are q chunks,
        # qc=1 FIRST: score strips 8..15 touch only q-cols [512:1024), so
        # they start as soon as the qc=1 half is evicted, overlapping the
        # qc=0 half and hiding the Q->scores transition.
        for qc in (1, 0):
            pss = [psA.tile([P, 512], F32, tag="ps", name="ps")
                   for _ in range(ET)]
            for dt in range(DT):
                for et in range(ET):
                    nc.tensor.matmul(
                        pss[et][:], lhsT=wq_t[:, dt, et * P:(et + 1) * P],
                        rhs=xqt[:, dt, qc * 512:(qc + 1) * 512],
                        start=(dt == 0), stop=(dt == DT - 1))
            for et in range(ET):
                nc.vector.tensor_copy(
                    qT[:, et, qc * 512:(qc + 1) * 512], pss[et][:])

    # ---------------- Phase B : attention (transposed scores) ----------
    # S^T[k, q] with keys on partitions, fp8 DoubleRow: each matmul
    # contracts an et PAIR into 64 psum partitions (one 64-key group).
    # exp(S^T) directly yields P^T -- the AV stationary operand.
    with body, ExitStack() as pb:
        stile = pb.enter_context(tc.tile_pool(name="st", bufs=1))
        sT = stile.tile([P, ST, QLOC], F32, tag="sT")   # S^T [k%128, kt, q]
        # per-slot P^T tiles so an early slot's AV only waits its own exp
        ptpool = pb.enter_context(tc.tile_pool(name="pt", bufs=QT))
        opool = pb.enter_context(tc.tile_pool(name="o", bufs=2))
        stpool = pb.enter_context(tc.tile_pool(name="stat", bufs=QT))
        psS = pb.enter_context(tc.tile_pool(name="psS", bufs=2, space="PSUM"))
        psAV = pb.enter_context(tc.tile_pool(name="psAV", bufs=3, space="PSUM"))
        psRS = pb.enter_context(tc.tile_pool(name="psRS", bufs=1, space="PSUM"))
        rs = psRS.tile([P, QT], F32, tag="rs")          # rowsum, col per slot

        pTs = {}
        for si, kt in enumerate(STRIP_ORDER):
            jq = JKT[kt] * P
            w = WKT[kt]
            if si >= 6:
                _load_mask(kt)
            for c0, cw in _chunks(w):
                ps = psS.tile([64, 2, 512], F32, tag="psS", name="ps")
                for kg in range(2):
                    for ep in range(ET // 2):
                        nc.tensor.matmul(
                            ps[:, kg, :cw],
                            lhsT=kT[:, 2 * ep:2 * ep + 2,
                                    kt * P + kg * 64:kt * P + kg * 64 + 64],
                            rhs=qT[:, 2 * ep:2 * ep + 2, jq + c0:jq + c0 + cw],
                            start=(ep == 0), stop=(ep == ET // 2 - 1),
                            perf_mode=DR)
                # mask-add eviction; kg1 lands on sT partitions 64:128
                for kg in range(2):
                    nc.vector.tensor_tensor(
                        sT[kg * 64:(kg + 1) * 64, kt, jq + c0:jq + c0 + cw],
                        ps[:, kg, :cw],
                        masks[kt][kg * 64:(kg + 1) * 64, c0:c0 + cw],
                        op=mybir.AluOpType.add)
            # fire exp for every slot whose strips are all processed now:
            # slots 0..3 at their last strip (kt = CNT-1 in the second
            # part), slots 4..7 once strip 7 closes the second part.
            ready = [i for i in range(QT) if CNT[i] == kt + 1] if kt < 8 else []
            if kt == 7:
                ready += [i for i in range(QT) if CNT[i] > 8]
            for i in ready:
                    pT_i = ptpool.tile([P, ST, P], BF16, tag="pt", name="pT_i")
                    nc.scalar.activation(
                        pT_i[:, 0:CNT[i], :],
                        sT[:, 0:CNT[i], i * P:(i + 1) * P],
                        mybir.ActivationFunctionType.Exp,
                        scale=INV_SQRT_D)
                    pTs[i] = pT_i

        for i in range(QT):
            ck = CNT[i]
            pT_i = pTs[i]
            psavs = [psAV.tile([P, 512], F32, tag="psAV", name="psavs")
                     for _ in range(2)]
            for kt in range(ck):
                lhsT = pT_i[:, kt, :]
                nc.tensor.matmul(rs[:, i:i + 1], lhsT=lhsT, rhs=ones[:],
                                 start=(kt == 0), stop=(kt == ck - 1))
                for ec in range(2):
                    nc.tensor.matmul(
                        psavs[ec][:], lhsT=lhsT,
                        rhs=v_sb[:, kt, ec * 512:(ec + 1) * 512],
                        start=(kt == 0), stop=(kt == ck - 1))

            recip = stpool.tile([P, 1], F32, tag="rc", name="recip")
            nc.vector.reciprocal(recip[:], rs[:, i:i + 1])
            for ec in range(2):
                o_t = opool.tile([P, 512], F32, tag="o", name="o_t")
                nc.scalar.activation(o_t[:], psavs[ec][:],
                                     mybir.ActivationFunctionType.Copy,
                                     scale=recip[:])
                nc.sync.dma_start(
                    out[i * P:(i + 1) * P, ec * 512:(ec + 1) * 512], o_t[:])


_COMPILED = None


def _get_compiled():
    global _COMPILED
    if _COMPILED is None:
        _COMPILED = _build()
    return _COMPILED


def _qrows(G):
    return np.concatenate([np.arange(g * P, (g + 1) * P) for g in G])


def _host_mask(G):
    # Transposed additive mask: for key-tile kt the active slots are the
    # q-suffix JKT[kt]..7; column c = (slot_index - JKT[kt])*128 + q_in_tile,
    # row r = key_in_tile.  0 where key <= query position, NEG otherwise.
    m = np.full((P, MASK_COLS), NEG, np.float32)
    for kt in range(ST):
        key = kt * P + np.arange(P)[:, None]
        qpos = np.concatenate(
            [G[i] * P + np.arange(P) for i in range(JKT[kt], QT)])[None, :]
        m[:, OFFKT[kt]:OFFKT[kt] + WKT[kt]] = np.where(
            key <= qpos, np.float32(0.0), np.float32(NEG))
    return m.astype(ml_dtypes.bfloat16)


def _host_in_maps(X, Wq, Wk, Wv):
    bf = ml_dtypes.bfloat16
    X = np.asarray(X, np.float32)
    wq = np.asarray(Wq, np.float32).astype(bf)
    wk = np.asarray(Wk, np.float32).astype(bf)
    wv = np.asarray(Wv, np.float32).astype(bf)
    masks = {0: _host_mask(G_A), 1: _host_mask(G_B)}
    qr = {0: _qrows(G_A), 1: _qrows(G_B)}
    in_maps = []
    for c in range(NCORES):
        b, h = divmod(c, 2)
        Xb = X[b]
        in_maps.append({
            "xt": np.ascontiguousarray(Xb[h * SLOC:(h + 1) * SLOC].T).astype(bf),
            "xqt": np.ascontiguousarray(Xb[qr[h]].T).astype(bf),
            "wq": wq, "wk": wk, "wv": wv,
            "mask": masks[h],
        })
    return in_maps, qr


def kernel(X, Wq, Wk, Wv, _trace=False):
    nc = _get_compiled()
    in_maps, qr = _host_in_maps(X, Wq, Wk, Wv)
    res = run_bass_kernel_spmd(nc, in_maps, core_ids=list(range(NCORES)),
                               trace=_trace)
    O = np.empty((B, S, D), np.float32)
    for c in range(NCORES):
        b, h = divmod(c, 2)
        O[b, qr[h]] = res.results[c]["out"]
    if _trace:
        kernel._last_exec_time_ns = res.exec_time_ns
        kernel._last_results = res
    return O
